# revision 1
# baseline (speedup 1.0000x reference)
"""CondConv (per-sample routed 3x3 conv) on 8 Trainium2 NeuronCores.

Reference computation (all fp32):
    gap     = mean(x, axis=(2,3))                    [B, CIN]
    routing = sigmoid(gap @ W_att.T + b_att)         [B, E]
    ker     = einsum('be,eoihw->boihw', routing, convs)
    out[b]  = conv2d(x[b], ker[b], stride 1, pad 1)  [B, COUT, 56, 56]

Sharding (B=32, COUT=256 across 8 cores): 4 core-pairs; pair p owns
samples 8p..8p+7 (batch data-parallel), and within a pair each core
computes one half of COUT (128 channels).

Per-core program (SPMD — same program, different data), bf16 datapath
(x, expert bank, mixed kernels, outputs bf16; accumulation, mix scratch
and routing fp32):
  - expert bank convsT [8e][2c][128cin, 9*128] bf16 resident in SBUF
  - per sample: DMA padded bf16 x -> GAP row sums via one ScalarE
    accum pass -> routing on DVE/GPSIMD/ScalarE (no TensorE, so the PE
    queue is pure conv) -> VectorE mixes the per-sample kernel with
    fused scalar_tensor_tensor (fp32 scratch, bf16 final write) -> conv
    as 2c*9shift*7tile accumulating bf16 matmuls (N=448, full PE rate,
    fp32 PSUM) -> ScalarE drains PSUM to bf16 -> DMA out.
  - engine budget per sample: PE 23.5us (the bottleneck), DVE
    routing+mix ~20.5us, ScalarE GAP+drains+sigmoid ~11.6us, so the
    software pipeline (mix b+2 overlaps conv b..b+1, input DMA 3 ahead)
    keeps PE stall-free after the prologue; ScalarE act tables are
    warmed at t~0 off the critical path.
"""

import numpy as np

B, CIN, H, W = 32, 256, 56, 56
COUT, KK, E = 256, 3, 8
HP, WP = H + 2, W + 2          # zero-padded input plane
PHW = HP * WP                  # 3364
NSH = KK * KK                  # 9 shifts
CHUNKS = 2                     # CIN = 2 * 128
MHALF = COUT // 2              # couts per core
ROWS_PER_TILE = 8              # output rows per matmul tile
NTILES = H // ROWS_PER_TILE    # 7
NFREE = ROWS_PER_TILE * W      # 448
NCORES = 8
SAMPLES_PER_CORE = B // (NCORES // 2)  # 8

_cached = {}


def _build_program():
    import concourse.bacc as bacc
    import concourse.bass_isa as bass_isa
    import concourse.mybir as mybir
    from concourse.tile import TileContext

    f32 = mybir.dt.float32
    bf16 = mybir.dt.bfloat16
    Alu = mybir.AluOpType
    Act = mybir.ActivationFunctionType

    nc = bacc.Bacc(None, target_bir_lowering=False)

    xpad_d = nc.declare_dram_parameter(
        "xpad", [SAMPLES_PER_CORE, CHUNKS, 128, PHW], bf16, isOutput=False)
    convsT_d = nc.declare_dram_parameter(
        "convsT", [E, CHUNKS, 128, NSH * 128], bf16, isOutput=False)
    watt_d = nc.declare_dram_parameter("watt", [CHUNKS, 128, E], f32, isOutput=False)
    battb_d = nc.declare_dram_parameter("battb", [128, E], f32, isOutput=False)
    out_d = nc.declare_dram_parameter(
        "out", [SAMPLES_PER_CORE, MHALF, H, W], bf16, isOutput=True)

    with TileContext(nc) as tc:
        with (
            tc.tile_pool(name="resident", bufs=1) as res_pool,
            tc.tile_pool(name="xp", bufs=3) as xp_pool,
            tc.tile_pool(name="kt", bufs=3) as kt_pool,
            tc.tile_pool(name="mixsc", bufs=2) as mixsc_pool,
            tc.tile_pool(name="small", bufs=3) as small_pool,
            tc.tile_pool(name="outsb", bufs=4) as out_pool,
            tc.tile_pool(name="cpsum", bufs=1, space="PSUM") as cps_pool,
        ):
            # ---- small resident tiles -------------------------------------
            watt_sb = []
            for c in range(CHUNKS):
                t = res_pool.tile([128, E], f32, name=f"watt{c}", tag=f"watt{c}")
                nc.sync.dma_start(out=t[:], in_=watt_d[c])
                watt_sb.append(t)
            battb_sb = res_pool.tile([128, E], f32, name="battb", tag="battb")
            nc.sync.dma_start(out=battb_sb[:], in_=battb_d[:])
            # warm the ScalarE activation tables (Copy+Sigmoid) at t~0 --
            # dep-free via a Pool memset -- so neither the routing sigmoid nor
            # the first drain pays the 1.3us table load on the critical path
            warm = small_pool.tile([128, 1], f32, name="warm", tag="warm")
            nc.gpsimd.memset(warm[:], 0.0)
            nc.scalar.activation(out=warm[:], in_=warm[:], func=Act.Copy)
            nc.scalar.activation(out=warm[:], in_=warm[:], func=Act.Sigmoid)
            # broadcast routing weights: scal[:, 8*b+e] = r_be on every partition
            scal_sb = res_pool.tile([128, SAMPLES_PER_CORE * E], f32,
                                    name="scal", tag="scal")

            convsT_sb = [[None] * CHUNKS for _ in range(E)]

            def emit_load_dma(b):
                """DMA padded input for sample b, split across 4 queues."""
                xp = []
                quarter = PHW // 4  # 841
                for c in range(CHUNKS):
                    t = xp_pool.tile([128, PHW], bf16, name=f"xp{c}", tag=f"xp{c}")
                    for j in range(4):
                        sl = slice(j * quarter, (j + 1) * quarter)
                        nc.sync.dma_start(out=t[:, sl], in_=xpad_d[b, c, :, sl])
                    xp.append(t)
                return xp

            def emit_load_gap(xp):
                """GAP pass: ScalarE in-place Copy whose accum_out yields
                the per-partition row sums (bf16 in, f32 accumulate). Split
                in halves so it overlaps the input DMA; off DVE so the mix
                owns the vector engine."""
                gq = []
                half = PHW // 2
                for c in range(CHUNKS):
                    for h in range(2):
                        sl = slice(h * half, (h + 1) * half)
                        g = small_pool.tile([128, 1], f32, name=f"gh{c}_{h}",
                                            tag=f"gh{c}_{h}")
                        nc.scalar.activation(out=xp[c][:, sl], in_=xp[c][:, sl],
                                             func=Act.Copy, accum_out=g[:])
                        gq.append(g)
                return gq

            def emit_load(b):
                xp = emit_load_dma(b)
                return xp, emit_load_gap(xp)

            def emit_routing(b, gs):
                """Routing for sample b on DVE/GPSIMD/ScalarE only.

                logits[e] = sum_cin gap[cin] * W_att[e,cin] / 3136 + b_att[e]
                (the 1/3136 is folded into watt host-side). Per-partition
                products on DVE, cross-partition sum on GPSIMD, sigmoid on
                ScalarE -- the TensorE queue stays pure conv.
                """
                gsum = []
                for c in range(CHUNKS):
                    g = small_pool.tile([128, 1], f32, name=f"gs{c}", tag=f"gs{c}")
                    nc.vector.tensor_add(out=g[:], in0=gs[2 * c][:],
                                         in1=gs[2 * c + 1][:])
                    gsum.append(g)
                t0 = small_pool.tile([128, E], f32, name="t0", tag="t0")
                nc.vector.tensor_scalar_mul(out=t0[:], in0=watt_sb[0][:],
                                            scalar1=gsum[0][:, 0:1])
                t1 = small_pool.tile([128, E], f32, name="t1", tag="t1")
                nc.vector.scalar_tensor_tensor(
                    out=t1[:], in0=watt_sb[1][:], scalar=gsum[1][:, 0:1],
                    in1=t0[:], op0=Alu.mult, op1=Alu.add)
                red = small_pool.tile([128, E], f32, name="red", tag="red")
                nc.gpsimd.partition_all_reduce(red[:], t1[:], channels=128,
                                               reduce_op=bass_isa.ReduceOp.add)
                red2 = small_pool.tile([128, E], f32, name="red2", tag="red2")
                nc.vector.tensor_add(out=red2[:], in0=red[:], in1=battb_sb[:])
                nc.scalar.activation(out=scal_sb[:, b * E:(b + 1) * E],
                                     in_=red2[:], func=Act.Sigmoid)

            def emit_mix_chunk(b, c, ngroups=1):
                """Mix chunk c of sample b's kernel on VectorE:
                kerT[c][cin, s*128+m] = sum_e r_be * convsT[e][c][cin, s*128+m]

                Accumulates in an fp32 scratch (the STT op gets no bf16 2x
                mode, so full precision is free) and converts to bf16 on the
                last expert's fused op. ngroups=3 splits the free dim into
                3-shift groups so the first group's mix (and the first conv
                matmuls) can run while later bank groups are still in DMA
                flight -- used for sample 0 on the prologue critical path.
                """
                k = kt_pool.tile([128, NSH * 128], bf16, name=f"kt{c}", tag=f"kt{c}")
                s = mixsc_pool.tile([128, NSH * 128], f32, name=f"ms{c}",
                                    tag=f"ms{c}")
                gw = (NSH * 128) // ngroups
                for g in range(ngroups):
                    sl = slice(g * gw, (g + 1) * gw)
                    nc.vector.tensor_scalar_mul(
                        out=s[:, sl], in0=convsT_sb[0][c][:, sl],
                        scalar1=scal_sb[:, b * E:b * E + 1])
                    for e in range(1, E - 1):
                        nc.vector.scalar_tensor_tensor(
                            out=s[:, sl], in0=convsT_sb[e][c][:, sl],
                            scalar=scal_sb[:, b * E + e:b * E + e + 1],
                            in1=s[:, sl], op0=Alu.mult, op1=Alu.add)
                    nc.vector.scalar_tensor_tensor(
                        out=k[:, sl], in0=convsT_sb[E - 1][c][:, sl],
                        scalar=scal_sb[:, b * E + E - 1:b * E + E],
                        in1=s[:, sl], op0=Alu.mult, op1=Alu.add)
                return k

            def emit_route_mix(b, gs):
                emit_routing(b, gs)
                return [emit_mix_chunk(b, c) for c in range(CHUNKS)]

            def emit_conv(b, xp, kt):
                """Conv for sample b: accumulate 2c*9shift into 7 PSUM tiles,
                then drain on VectorE and store."""
                cps = [cps_pool.tile([128, NFREE], f32, name=f"cps{n}",
                                     tag=f"cps{n}", bufs=2 if n == 0 else 1)
                       for n in range(NTILES)]
                for c in range(CHUNKS):
                    x3 = xp[c].rearrange("p (r q) -> p r q", q=WP)
                    for s in range(NSH):
                        dh, dw = s // KK, s % KK
                        lhsT = kt[c][:, s * 128:(s + 1) * 128]
                        first = (c == 0 and s == 0)
                        last = (c == CHUNKS - 1 and s == NSH - 1)
                        for n in range(NTILES):
                            rhs = x3[:, n * ROWS_PER_TILE + dh:
                                     n * ROWS_PER_TILE + dh + ROWS_PER_TILE,
                                     dw:dw + W]
                            nc.tensor.matmul(cps[n][:], lhsT, rhs,
                                             start=first, stop=last)
                # all drains on ScalarE: VectorE budget (routing+mix
                # ~20.5us/sample) must stay under PE's 23.5us/sample, while
                # ScalarE only carries GAP+drains+sigmoid (~11.6us/sample)
                for n in range(NTILES):
                    o = out_pool.tile([128, NFREE], bf16, name="osb", tag="osb")
                    nc.scalar.activation(out=o[:], in_=cps[n][:], func=Act.Copy)
                    nc.sync.dma_start(
                        out=out_d[b, :, n * ROWS_PER_TILE:(n + 1) * ROWS_PER_TILE, :],
                        in_=o[:])

            # ---- software-pipelined emission ------------------------------
            # All loads ride the same HWDGE rings as the conv bank, so queue
            # FIFO order enforces: xp(0), bank chunk0, xp(1), bank chunk1,
            # xp(2), then steady-state loads 3 samples ahead. routing+mix of
            # sample b+2 is emitted after conv(b) so the mix overlaps conv(b)
            # on VectorE, with drains at the queue head for prompt PSUM
            # recycling. The prologue hand-orders sample 0/1 mix chunks
            # around the bank-chunk arrivals.
            S = SAMPLES_PER_CORE

            def emit_bank_chunk(c, ngroups=1):
                """ngroups=3 orders the chunk's DMAs group-major (all experts'
                shift-group g before group g+1) so a grouped mix can start on
                group 0 while groups 1-2 are still loading."""
                for e in range(E):
                    t = res_pool.tile([128, NSH * 128], bf16,
                                      name=f"cv_{e}_{c}", tag=f"cv_{e}_{c}")
                    convsT_sb[e][c] = t
                gw = (NSH * 128) // ngroups
                for g in range(ngroups):
                    sl = slice(g * gw, (g + 1) * gw)
                    for e in range(E):
                        nc.sync.dma_start(out=convsT_sb[e][c][:, sl],
                                          in_=convsT_d[e, c, :, sl])

            loads = {0: emit_load(0)}
            emit_bank_chunk(0)
            emit_routing(0, loads[0][1])
            kt0c0 = emit_mix_chunk(0, 0)
            xp1 = emit_load_dma(1)
            emit_bank_chunk(1)
            kt0c1 = emit_mix_chunk(0, 1)
            kts = {0: [kt0c0, kt0c1]}
            loads[1] = (xp1, emit_load_gap(xp1))
            emit_routing(1, loads[1][1])
            kt1c0 = emit_mix_chunk(1, 0)
            loads[2] = emit_load(2)
            emit_conv(0, loads.pop(0)[0], kts.pop(0))
            kts[1] = [kt1c0, emit_mix_chunk(1, 1)]
            for b in range(1, S):
                if b + 1 < S and b + 1 not in kts:
                    kts[b + 1] = emit_route_mix(b + 1, loads[b + 1][1])
                if b + 2 < S:
                    loads[b + 2] = emit_load(b + 2)
                emit_conv(b, loads.pop(b)[0], kts.pop(b))

    nc.compile()
    return nc


def _prep_core_inputs(x, convs, W_att, b_att):
    """Host-side shard/layout prep. Returns list of 8 per-core input dicts."""
    import ml_dtypes
    f32 = np.float32
    bf16 = ml_dtypes.bfloat16
    # padded input, cin split into 2 chunks of 128
    xpad = np.zeros((B, CHUNKS, 128, HP, WP), dtype=bf16)
    xpad[:, :, :, 1:H + 1, 1:W + 1] = np.ascontiguousarray(x, dtype=f32).reshape(
        B, CHUNKS, 128, H, W).astype(bf16)
    xpad = xpad.reshape(B, CHUNKS, 128, PHW)

    # convsT[half][e, c, cin, s*128 + m] = convs[e, half*128+m, c*128+cin, kh, kw]
    cv = np.ascontiguousarray(convs, dtype=f32).reshape(E, 2, MHALF, CHUNKS, 128, NSH)
    convsT_halves = [
        np.ascontiguousarray(cv[:, h].transpose(0, 2, 3, 4, 1).reshape(
            E, CHUNKS, 128, NSH * 128)).astype(bf16)
        for h in range(2)
    ]

    watt = np.ascontiguousarray(
        (np.asarray(W_att, dtype=f32).T / f32(H * W)).reshape(CHUNKS, 128, E))
    battb = np.ascontiguousarray(
        np.broadcast_to(np.asarray(b_att, dtype=f32), (128, E)))

    in_maps = []
    for k in range(NCORES):
        pair, half = k // 2, k % 2
        sl = slice(pair * SAMPLES_PER_CORE, (pair + 1) * SAMPLES_PER_CORE)
        in_maps.append({
            "xpad": np.ascontiguousarray(xpad[sl]),
            "convsT": convsT_halves[half],
            "watt": watt,
            "battb": battb,
        })
    return in_maps


def _assemble_output(results):
    out = np.empty((B, COUT, H, W), dtype=np.float32)
    for k in range(NCORES):
        pair, half = k // 2, k % 2
        sl = slice(pair * SAMPLES_PER_CORE, (pair + 1) * SAMPLES_PER_CORE)
        out[sl, half * MHALF:(half + 1) * MHALF] = np.asarray(
            results[k]["out"], dtype=np.float32)
    return out


def kernel(x, convs, W_att, b_att):
    from concourse.bass_utils import run_bass_kernel_spmd

    if "nc" not in _cached:
        _cached["nc"] = _build_program()
    in_maps = _prep_core_inputs(x, convs, W_att, b_att)
    res = run_bass_kernel_spmd(_cached["nc"], in_maps, core_ids=list(range(NCORES)))
    return _assemble_output(res.results)



# revision 2
# speedup vs baseline: 1.0175x; 1.0175x over previous
"""CondConv (per-sample routed 3x3 conv) on 8 Trainium2 NeuronCores.

Reference computation (all fp32):
    gap     = mean(x, axis=(2,3))                    [B, CIN]
    routing = sigmoid(gap @ W_att.T + b_att)         [B, E]
    ker     = einsum('be,eoihw->boihw', routing, convs)
    out[b]  = conv2d(x[b], ker[b], stride 1, pad 1)  [B, COUT, 56, 56]

Sharding (B=32, COUT=256 across 8 cores): 4 core-pairs; pair p owns
samples 8p..8p+7 (batch data-parallel), and within a pair each core
computes one half of COUT (128 channels).

Per-core program (SPMD), bf16 datapath, fp32 PSUM accumulation:
  - expert bank resident in ONE SBUF tile [128cin, E*2304] so the whole
    bank loads as 6 large DMAs (vs 16 small ones); DMA order is
    xp(0) -> bank -> xp(1) -> xp(2) so sample 0's GAP/routing overlaps
    the bank load and the first matmul fires as early as possible.
  - routing on ScalarE(GAP accum + sigmoid)/DVE/GPSIMD; TensorE queue
    stays pure conv.
  - kernel mix on DVE as 8 tensor_scalar mults (4x bf16 mode) + 7
    tensor_tensor adds (2x bf16 mode) ~15.6us/sample, well under PE's
    ~25us/sample -- STT (no fast mode) would be 20.4us and starve the
    pipeline during the prologue. Samples 0/1 mix in column groups so
    conv(0) starts after only the first group.
  - conv: per sample 2chunk*9shift*7tile accumulating bf16 matmuls
    (N=448) into 7 PSUM tiles drawn from an 8-buffer rotating pool;
    the last accumulation round interleaves drains (ScalarE/DVE
    ping-pong) right behind each tile's final matmul so the next
    sample's matmuls never wait on PSUM recycling.
  - output: drains collect into one [128, 3136] SBUF tile, stored with
    a single DMA per sample (last sample: per-tile DMAs to cut the
    epilogue tail).
"""

import numpy as np

B, CIN, H, W = 32, 256, 56, 56
COUT, KK, E = 256, 3, 8
HP, WP = H + 2, W + 2          # zero-padded input plane
PHW = HP * WP                  # 3364
NSH = KK * KK                  # 9 shifts
CHUNKS = 2                     # CIN = 2 * 128
MHALF = COUT // 2              # couts per core
ROWS_PER_TILE = 8              # output rows per matmul tile
NTILES = H // ROWS_PER_TILE    # 7
NFREE = ROWS_PER_TILE * W      # 448
NCORES = 8
SAMPLES_PER_CORE = B // (NCORES // 2)  # 8
KCOLS = NSH * 128              # 1152 kernel cols per chunk
KWID = CHUNKS * KCOLS          # 2304 kernel cols per sample

_cached = {}


def _build_program():
    import concourse.bacc as bacc
    import concourse.bass_isa as bass_isa
    import concourse.mybir as mybir
    from concourse.tile import TileContext

    f32 = mybir.dt.float32
    bf16 = mybir.dt.bfloat16
    Alu = mybir.AluOpType
    Act = mybir.ActivationFunctionType

    nc = bacc.Bacc(None, target_bir_lowering=False)

    S = SAMPLES_PER_CORE
    xpad_d = nc.declare_dram_parameter(
        "xpad", [S, CHUNKS, 128, PHW], bf16, isOutput=False)
    bank_d = nc.declare_dram_parameter(
        "bank", [CHUNKS, 3, 128, E, 3 * 128], bf16, isOutput=False)
    watbat_d = nc.declare_dram_parameter("watbat", [128, 3 * E], f32,
                                         isOutput=False)
    out_d = nc.declare_dram_parameter(
        "out", [S, MHALF, H, W], bf16, isOutput=True)

    with TileContext(nc) as tc:
        with (
            tc.tile_pool(name="resident", bufs=1) as res_pool,
            tc.tile_pool(name="xp", bufs=3) as xp_pool,
            tc.tile_pool(name="kt", bufs=3) as kt_pool,
            tc.tile_pool(name="mixt", bufs=1) as t_pool,
            tc.tile_pool(name="mixu", bufs=2) as u_pool,
            tc.tile_pool(name="small", bufs=3) as small_pool,
            tc.tile_pool(name="outsb", bufs=2) as out_pool,
            tc.tile_pool(name="cpsum", bufs=8, space="PSUM") as cps_pool,
        ):
            # ---- small resident tiles -------------------------------------
            watbat_sb = res_pool.tile([128, 3 * E], f32, name="watbat",
                                      tag="watbat")
            nc.sync.dma_start(out=watbat_sb[:], in_=watbat_d[:])
            # warm the ScalarE activation tables (Copy+Sigmoid) at t~0 --
            # dep-free via a GpSimd memset -- so neither the routing sigmoid
            # nor the first drain pays the table load on the critical path
            warm = small_pool.tile([128, 1], f32, name="warm", tag="warm")
            nc.gpsimd.memset(warm[:], 0.0)
            nc.scalar.activation(out=warm[:], in_=warm[:], func=Act.Copy)
            nc.scalar.activation(out=warm[:], in_=warm[:], func=Act.Sigmoid)
            # broadcast routing weights: scal[:, 8*b+e] = r_be on every
            # partition
            scal_sb = res_pool.tile([128, S * E], f32, name="scal", tag="scal")
            # whole expert bank in one tile: col = e*2304 + c*1152 + s*128 + m
            bank_sb = res_pool.tile([128, E * KWID], bf16, name="bank",
                                    tag="bank")

            def emit_bank_dma():
                v = bank_sb.rearrange("p (e q) -> p e q", e=E)
                for c in range(CHUNKS):
                    for g in range(3):
                        a = c * KCOLS + g * 384
                        nc.sync.dma_start(out=v[:, :, a:a + 384],
                                          in_=bank_d[c, g])

            def emit_xp(b, split=1):
                xp = []
                for c in range(CHUNKS):
                    t = xp_pool.tile([128, PHW], bf16, name=f"xp{c}",
                                     tag=f"xp{c}")
                    if split == 1:
                        nc.sync.dma_start(out=t[:], in_=xpad_d[b, c])
                    else:
                        piece = PHW // split
                        for j in range(split):
                            sl = slice(j * piece, (j + 1) * piece)
                            nc.sync.dma_start(out=t[:, sl],
                                              in_=xpad_d[b, c, :, sl])
                    xp.append(t)
                return xp

            def emit_gap(b, xp, split=1):
                """GAP via ScalarE in-place Copy whose accum_out yields the
                per-partition row sums (bf16 in, f32 accumulate)."""
                gq = []
                for c in range(CHUNKS):
                    pieces = []
                    w = PHW // split
                    for j in range(split):
                        sl = slice(j * w, (j + 1) * w)
                        g = small_pool.tile([128, 1], f32, name=f"gh{c}_{j}",
                                            tag=f"gh{c}_{j}")
                        nc.scalar.activation(out=xp[c][:, sl],
                                             in_=xp[c][:, sl],
                                             func=Act.Copy, accum_out=g[:])
                        pieces.append(g)
                    gq.append(pieces)
                return gq

            def emit_routing(b, gq):
                """logits[e] = sum_cin gap[cin]*W_att[e,cin]/3136 + b_att[e]
                (1/3136 folded into watbat host-side); sigmoid -> scal_sb."""
                gsum = []
                for c in range(CHUNKS):
                    if len(gq[c]) == 1:
                        gsum.append(gq[c][0])
                    else:
                        g = small_pool.tile([128, 1], f32, name=f"gs{c}",
                                            tag=f"gs{c}")
                        nc.vector.tensor_add(out=g[:], in0=gq[c][0][:],
                                             in1=gq[c][1][:])
                        gsum.append(g)
                t0 = small_pool.tile([128, E], f32, name="t0", tag="t0")
                nc.vector.tensor_scalar_mul(out=t0[:],
                                            in0=watbat_sb[:, 0:E],
                                            scalar1=gsum[0][:, 0:1])
                t1 = small_pool.tile([128, E], f32, name="t1", tag="t1")
                nc.vector.scalar_tensor_tensor(
                    out=t1[:], in0=watbat_sb[:, E:2 * E],
                    scalar=gsum[1][:, 0:1],
                    in1=t0[:], op0=Alu.mult, op1=Alu.add)
                red = small_pool.tile([128, E], f32, name="red", tag="red")
                nc.gpsimd.partition_all_reduce(red[:], t1[:], channels=128,
                                               reduce_op=bass_isa.ReduceOp.add)
                red2 = small_pool.tile([128, E], f32, name="red2", tag="red2")
                nc.vector.tensor_add(out=red2[:], in0=red[:],
                                     in1=watbat_sb[:, 2 * E:3 * E])
                nc.scalar.activation(out=scal_sb[:, b * E:(b + 1) * E],
                                     in_=red2[:], func=Act.Sigmoid)

            def emit_mix(b, groups):
                """Mix sample b's kernel on DVE with fast bf16 modes:
                t_e = r_be * bank_e   (tensor_scalar, 4x bf16)
                kt  = tree-sum(t_e)   (tensor_tensor adds, 2x bf16)
                `groups` splits the 2304 free cols so early groups unblock
                conv while later bank columns may still be in DMA flight.
                """
                kt = kt_pool.tile([128, KWID], bf16, name="kt", tag="kt")
                ts = [t_pool.tile([128, KWID], bf16, name=f"t{e}",
                                  tag=f"t{e}") for e in range(E)]
                u = [u_pool.tile([128, KWID], bf16, name=f"u{i}", tag=f"u{i}")
                     for i in range(6)]
                r = lambda e: scal_sb[:, b * E + e:b * E + e + 1]
                for (a, z) in groups:
                    for e in range(E):
                        nc.vector.tensor_scalar_mul(
                            out=ts[e][:, a:z],
                            in0=bank_sb[:, e * KWID + a:e * KWID + z],
                            scalar1=r(e))
                    pairs = [(u[0], ts[0], ts[1]), (u[1], ts[2], ts[3]),
                             (u[2], ts[4], ts[5]), (u[3], ts[6], ts[7]),
                             (u[4], u[0], u[1]), (u[5], u[2], u[3]),
                             (kt, u[4], u[5])]
                    for (o, i0, i1) in pairs:
                        nc.vector.tensor_add(out=o[:, a:z], in0=i0[:, a:z],
                                             in1=i1[:, a:z])
                return kt

            def emit_conv(b, xp, kt, pingpong, last):
                """Conv for sample b: accumulate 2c*9shift into 7 PSUM tiles
                from the 8-buffer rotating pool. The final round interleaves
                drains right behind each tile's stop-matmul so the next
                sample's start-matmuls find their PSUM banks already free."""
                cps = [cps_pool.tile([128, NFREE], f32, name="cps", tag="cps")
                       for _ in range(NTILES)]
                x3 = [xp[c].rearrange("p (r q) -> p r q", q=WP)
                      for c in range(CHUNKS)]
                if not last:
                    o = out_pool.tile([128, H * W], bf16, name="osb",
                                      tag="osb")
                for c in range(CHUNKS):
                    for s in range(NSH):
                        if c == CHUNKS - 1 and s == NSH - 1:
                            continue
                        dh, dw = s // KK, s % KK
                        lhsT = kt[:, c * KCOLS + s * 128:
                                  c * KCOLS + (s + 1) * 128]
                        first = (c == 0 and s == 0)
                        for n in range(NTILES):
                            rhs = x3[c][:, n * ROWS_PER_TILE + dh:
                                        n * ROWS_PER_TILE + dh + ROWS_PER_TILE,
                                        dw:dw + W]
                            nc.tensor.matmul(cps[n][:], lhsT, rhs,
                                             start=first, stop=False)
                c, s = CHUNKS - 1, NSH - 1
                dh, dw = s // KK, s % KK
                lhsT = kt[:, c * KCOLS + s * 128:c * KCOLS + (s + 1) * 128]
                for n in range(NTILES):
                    rhs = x3[c][:, n * ROWS_PER_TILE + dh:
                                n * ROWS_PER_TILE + dh + ROWS_PER_TILE,
                                dw:dw + W]
                    nc.tensor.matmul(cps[n][:], lhsT, rhs,
                                     start=False, stop=True)
                    use_dve = pingpong and (n % 2 == 1)
                    if last:
                        ot = out_pool.tile([128, NFREE], bf16, name="osbl",
                                           tag="osbl")
                        dst = ot[:]
                    else:
                        dst = o[:, n * NFREE:(n + 1) * NFREE]
                    if use_dve:
                        nc.vector.tensor_copy(out=dst, in_=cps[n][:])
                    else:
                        nc.scalar.activation(out=dst, in_=cps[n][:],
                                             func=Act.Copy)
                    if last:
                        nc.sync.dma_start(
                            out=out_d[b, :,
                                      n * ROWS_PER_TILE:(n + 1) * ROWS_PER_TILE,
                                      :],
                            in_=ot[:])
                if not last:
                    nc.sync.dma_start(out=out_d[b], in_=o[:])

            # ---- software-pipelined emission ------------------------------
            # Sync-queue FIFO order gives DMA priority: watbat, xp(0), bank,
            # xp(1), xp(2), then per-iteration xp prefetch 3 samples ahead
            # behind each sample's output store.
            xps, gqs, kts = {}, {}, {}
            xps[0] = emit_xp(0, split=2)
            gqs[0] = emit_gap(0, xps[0], split=2)
            emit_bank_dma()
            emit_routing(0, gqs[0])
            kts[0] = emit_mix(0, [(0, 512), (512, 1152), (1152, KWID)])
            xps[1] = emit_xp(1)
            gqs[1] = emit_gap(1, xps[1])
            emit_routing(1, gqs[1])
            kts[1] = emit_mix(1, [(0, 1152), (1152, KWID)])
            xps[2] = emit_xp(2)
            gqs[2] = emit_gap(2, xps[2])
            for b in range(S):
                emit_conv(b, xps.pop(b), kts.pop(b), pingpong=(b > 0),
                          last=(b == S - 1))
                if b + 2 < S:
                    emit_routing(b + 2, gqs.pop(b + 2))
                    kts[b + 2] = emit_mix(b + 2, [(0, KWID)])
                if b + 3 < S:
                    xps[b + 3] = emit_xp(b + 3)
                    gqs[b + 3] = emit_gap(b + 3, xps[b + 3])

    nc.compile()
    return nc


def _prep_core_inputs(x, convs, W_att, b_att):
    """Host-side shard/layout prep. Returns list of 8 per-core input dicts."""
    import ml_dtypes
    f32 = np.float32
    bf16 = ml_dtypes.bfloat16
    # padded input, cin split into 2 chunks of 128
    xpad = np.zeros((B, CHUNKS, 128, HP, WP), dtype=bf16)
    xpad[:, :, :, 1:H + 1, 1:W + 1] = np.ascontiguousarray(x, dtype=f32).reshape(
        B, CHUNKS, 128, H, W).astype(bf16)
    xpad = xpad.reshape(B, CHUNKS, 128, PHW)

    # bank[half][c, g, p, e, kw*128 + m] = convs[e, half*128+m, c*128+p, g, kw]
    cv = np.ascontiguousarray(convs, dtype=f32).reshape(
        E, 2, MHALF, CHUNKS, 128, KK, KK)
    bank_halves = [
        np.ascontiguousarray(cv[:, h].transpose(2, 4, 3, 0, 5, 1).reshape(
            CHUNKS, 3, 128, E, 3 * 128)).astype(bf16)
        for h in range(2)
    ]

    watt = (np.asarray(W_att, dtype=f32).T / f32(H * W)).astype(f32)  # [CIN, E]
    watbat = np.empty((128, 3 * E), dtype=f32)
    watbat[:, 0:E] = watt[:128]
    watbat[:, E:2 * E] = watt[128:]
    watbat[:, 2 * E:3 * E] = np.broadcast_to(np.asarray(b_att, dtype=f32),
                                             (128, E))

    in_maps = []
    for k in range(NCORES):
        pair, half = k // 2, k % 2
        sl = slice(pair * SAMPLES_PER_CORE, (pair + 1) * SAMPLES_PER_CORE)
        in_maps.append({
            "xpad": np.ascontiguousarray(xpad[sl]),
            "bank": bank_halves[half],
            "watbat": watbat,
        })
    return in_maps


def _assemble_output(results):
    out = np.empty((B, COUT, H, W), dtype=np.float32)
    for k in range(NCORES):
        pair, half = k // 2, k % 2
        sl = slice(pair * SAMPLES_PER_CORE, (pair + 1) * SAMPLES_PER_CORE)
        out[sl, half * MHALF:(half + 1) * MHALF] = np.asarray(
            results[k]["out"], dtype=np.float32)
    return out


def kernel(x, convs, W_att, b_att):
    from concourse.bass_utils import run_bass_kernel_spmd

    if "nc" not in _cached:
        _cached["nc"] = _build_program()
    in_maps = _prep_core_inputs(x, convs, W_att, b_att)
    res = run_bass_kernel_spmd(_cached["nc"], in_maps, core_ids=list(range(NCORES)))
    return _assemble_output(res.results)


# revision 7
# speedup vs baseline: 1.0514x; 1.0333x over previous
"""CondConv (per-sample routed 3x3 conv) on 8 Trainium2 NeuronCores.

Reference computation (all fp32):
    gap     = mean(x, axis=(2,3))                    [B, CIN]
    routing = sigmoid(gap @ W_att.T + b_att)         [B, E]
    ker     = einsum('be,eoihw->boihw', routing, convs)
    out[b]  = conv2d(x[b], ker[b], stride 1, pad 1)  [B, COUT, 56, 56]

Sharding (B=32, COUT=256 across 8 cores): 4 core-pairs; pair p owns
samples 8p..8p+7 (batch data-parallel), and within a pair each core
computes one half of COUT (128 channels).

Per-core program (SPMD), bf16 datapath, fp32 PSUM accumulation:
  - expert bank resident in ONE SBUF tile [128cin, E*2304] so the whole
    bank loads as 6 large DMAs (vs 16 small ones); DMA order is
    xp(0) -> bank -> xp(1) -> xp(2) so sample 0's GAP/routing overlaps
    the bank load and the first matmul fires as early as possible.
  - routing on ScalarE(GAP accum + sigmoid)/DVE/GPSIMD; TensorE queue
    stays pure conv.
  - kernel mix on DVE as 8 tensor_scalar mults (4x bf16 mode) + 7
    tensor_tensor adds (2x bf16 mode) ~15.6us/sample, well under PE's
    ~25us/sample -- STT (no fast mode) would be 20.4us and starve the
    pipeline during the prologue. Samples 0/1 mix in column groups so
    conv(0) starts after only the first group.
  - conv: per sample 2chunk*9shift*7tile accumulating bf16 matmuls
    (N=448) into 7 PSUM tiles drawn from an 8-buffer rotating pool;
    the last accumulation round interleaves drains (ScalarE/DVE
    ping-pong) right behind each tile's final matmul so the next
    sample's matmuls never wait on PSUM recycling.
  - output: drains collect into one [128, 3136] SBUF tile, stored with
    a single DMA per sample (last sample: per-tile DMAs to cut the
    epilogue tail).
"""

import numpy as np

B, CIN, H, W = 32, 256, 56, 56
COUT, KK, E = 256, 3, 8
HP, WP = H + 2, W + 2          # zero-padded input plane
PHW = HP * WP                  # 3364
NSH = KK * KK                  # 9 shifts
CHUNKS = 2                     # CIN = 2 * 128
MHALF = COUT // 2              # couts per core
ROWS_PER_TILE = 8              # output rows per matmul tile
NTILES = H // ROWS_PER_TILE    # 7
NFREE = ROWS_PER_TILE * W      # 448
NCORES = 8
SAMPLES_PER_CORE = B // (NCORES // 2)  # 8
KCOLS = NSH * 128              # 1152 kernel cols per chunk
KWID = CHUNKS * KCOLS          # 2304 kernel cols per sample

_cached = {}


def _build_program():
    import concourse.bacc as bacc
    import concourse.bass_isa as bass_isa
    import concourse.mybir as mybir
    from concourse.tile import TileContext

    f32 = mybir.dt.float32
    bf16 = mybir.dt.bfloat16
    Alu = mybir.AluOpType
    Act = mybir.ActivationFunctionType

    nc = bacc.Bacc(None, target_bir_lowering=False)

    S = SAMPLES_PER_CORE
    xpad_d = nc.declare_dram_parameter(
        "xpad", [S, CHUNKS, 128, PHW], bf16, isOutput=False)
    bank_d = nc.declare_dram_parameter(
        "bank", [CHUNKS, 3, 128, E, 3 * 128], bf16, isOutput=False)
    watbat_d = nc.declare_dram_parameter("watbat", [128, 3 * E], f32,
                                         isOutput=False)
    out_d = nc.declare_dram_parameter(
        "out", [S, MHALF, H, W], bf16, isOutput=True)

    with TileContext(nc) as tc:
        with (
            tc.tile_pool(name="resident", bufs=1) as res_pool,
            tc.tile_pool(name="xp", bufs=3) as xp_pool,
            tc.tile_pool(name="kt", bufs=3) as kt_pool,
            tc.tile_pool(name="mixt", bufs=1) as t_pool,
            tc.tile_pool(name="mixu", bufs=2) as u_pool,
            tc.tile_pool(name="small", bufs=3) as small_pool,
            tc.tile_pool(name="outsb", bufs=2) as out_pool,
            tc.tile_pool(name="cpsum", bufs=8, space="PSUM") as cps_pool,
        ):
            # ---- small resident tiles -------------------------------------
            watbat_sb = res_pool.tile([128, 3 * E], f32, name="watbat",
                                      tag="watbat")
            nc.sync.dma_start(out=watbat_sb[:], in_=watbat_d[:])
            # warm the ScalarE activation tables (Copy+Sigmoid) at t~0 --
            # dep-free via a GpSimd memset -- so neither the routing sigmoid
            # nor the first drain pays the table load on the critical path
            warm = small_pool.tile([128, 1], f32, name="warm", tag="warm")
            nc.gpsimd.memset(warm[:], 0.0)
            nc.scalar.activation(out=warm[:], in_=warm[:], func=Act.Copy)
            nc.scalar.activation(out=warm[:], in_=warm[:], func=Act.Sigmoid)
            # broadcast routing weights: scal[:, 8*b+e] = r_be on every
            # partition
            scal_sb = res_pool.tile([128, S * E], f32, name="scal", tag="scal")
            # whole expert bank in one tile: col = e*2304 + c*1152 + s*128 + m
            bank_sb = res_pool.tile([128, E * KWID], bf16, name="bank",
                                    tag="bank")

            def emit_bank_dma():
                v = bank_sb.rearrange("p (e q) -> p e q", e=E)
                for c in range(CHUNKS):
                    for g in range(3):
                        a = c * KCOLS + g * 384
                        nc.sync.dma_start(out=v[:, :, a:a + 384],
                                          in_=bank_d[c, g])

            def emit_xp(b, split=1):
                xp = []
                for c in range(CHUNKS):
                    t = xp_pool.tile([128, PHW], bf16, name=f"xp{c}",
                                     tag=f"xp{c}")
                    if split == 1:
                        nc.sync.dma_start(out=t[:], in_=xpad_d[b, c])
                    else:
                        piece = PHW // split
                        for j in range(split):
                            sl = slice(j * piece, (j + 1) * piece)
                            nc.sync.dma_start(out=t[:, sl],
                                              in_=xpad_d[b, c, :, sl])
                    xp.append(t)
                return xp

            def emit_gap(b, xp, split=1):
                """GAP via ScalarE in-place Copy whose accum_out yields the
                per-partition row sums (bf16 in, f32 accumulate)."""
                gq = []
                for c in range(CHUNKS):
                    pieces = []
                    w = PHW // split
                    for j in range(split):
                        sl = slice(j * w, (j + 1) * w)
                        g = small_pool.tile([128, 1], f32, name=f"gh{c}_{j}",
                                            tag=f"gh{c}_{j}")
                        nc.scalar.activation(out=xp[c][:, sl],
                                             in_=xp[c][:, sl],
                                             func=Act.Copy, accum_out=g[:])
                        pieces.append(g)
                    gq.append(pieces)
                return gq

            def emit_routing(b, gq):
                """logits[e] = sum_cin gap[cin]*W_att[e,cin]/3136 + b_att[e]
                (1/3136 folded into watbat host-side; b_att/128 lives in
                watbat[:, 2E:3E] so the partition all-reduce sums it back to
                b_att -- no separate bias add); sigmoid -> scal_sb."""
                gsum = []
                for c in range(CHUNKS):
                    if len(gq[c]) == 1:
                        gsum.append(gq[c][0])
                    else:
                        g = small_pool.tile([128, 1], f32, name=f"gs{c}",
                                            tag=f"gs{c}")
                        nc.vector.tensor_add(out=g[:], in0=gq[c][0][:],
                                             in1=gq[c][1][:])
                        gsum.append(g)
                t0 = small_pool.tile([128, E], f32, name="t0", tag="t0")
                nc.vector.scalar_tensor_tensor(
                    out=t0[:], in0=watbat_sb[:, 0:E],
                    scalar=gsum[0][:, 0:1],
                    in1=watbat_sb[:, 2 * E:3 * E], op0=Alu.mult, op1=Alu.add)
                t1 = small_pool.tile([128, E], f32, name="t1", tag="t1")
                nc.vector.scalar_tensor_tensor(
                    out=t1[:], in0=watbat_sb[:, E:2 * E],
                    scalar=gsum[1][:, 0:1],
                    in1=t0[:], op0=Alu.mult, op1=Alu.add)
                red = small_pool.tile([128, E], f32, name="red", tag="red")
                nc.gpsimd.partition_all_reduce(red[:], t1[:], channels=128,
                                               reduce_op=bass_isa.ReduceOp.add)
                nc.scalar.activation(out=scal_sb[:, b * E:(b + 1) * E],
                                     in_=red[:], func=Act.Sigmoid)

            def emit_mix(b, groups):
                """Mix sample b's kernel on DVE with fast bf16 modes:
                t_e = r_be * bank_e   (tensor_scalar, 4x bf16)
                kt  = tree-sum(t_e)   (tensor_tensor adds, 2x bf16)
                `groups` splits the 2304 free cols so early groups unblock
                conv while later bank columns may still be in DMA flight.
                """
                kt = kt_pool.tile([128, KWID], bf16, name="kt", tag="kt")
                ts = [t_pool.tile([128, KWID], bf16, name=f"t{e}",
                                  tag=f"t{e}") for e in range(E)]
                u = [u_pool.tile([128, KWID], bf16, name=f"u{i}", tag=f"u{i}")
                     for i in range(6)]
                r = lambda e: scal_sb[:, b * E + e:b * E + e + 1]
                for (a, z) in groups:
                    for e in range(E):
                        nc.vector.tensor_scalar_mul(
                            out=ts[e][:, a:z],
                            in0=bank_sb[:, e * KWID + a:e * KWID + z],
                            scalar1=r(e))
                    pairs = [(u[0], ts[0], ts[1]), (u[1], ts[2], ts[3]),
                             (u[2], ts[4], ts[5]), (u[3], ts[6], ts[7]),
                             (u[4], u[0], u[1]), (u[5], u[2], u[3]),
                             (kt, u[4], u[5])]
                    for (o, i0, i1) in pairs:
                        nc.vector.tensor_add(out=o[:, a:z], in0=i0[:, a:z],
                                             in1=i1[:, a:z])
                return kt

            def emit_conv(b, xp, kt, pingpong, last):
                """Conv for sample b: accumulate 2c*9shift into 7 PSUM tiles
                from the 8-buffer rotating pool. The final round interleaves
                drains right behind each tile's stop-matmul so the next
                sample's start-matmuls find their PSUM banks already free."""
                cps = [cps_pool.tile([128, NFREE], f32, name="cps", tag="cps")
                       for _ in range(NTILES)]
                x3 = [xp[c].rearrange("p (r q) -> p r q", q=WP)
                      for c in range(CHUNKS)]
                o = out_pool.tile([128, H * W], bf16, name="osb", tag="osb")
                for c in range(CHUNKS):
                    for s in range(NSH):
                        if c == CHUNKS - 1 and s == NSH - 1:
                            continue
                        dh, dw = s // KK, s % KK
                        lhsT = kt[:, c * KCOLS + s * 128:
                                  c * KCOLS + (s + 1) * 128]
                        first = (c == 0 and s == 0)
                        for n in range(NTILES):
                            rhs = x3[c][:, n * ROWS_PER_TILE + dh:
                                        n * ROWS_PER_TILE + dh + ROWS_PER_TILE,
                                        dw:dw + W]
                            nc.tensor.matmul(cps[n][:], lhsT, rhs,
                                             start=first, stop=False)
                c, s = CHUNKS - 1, NSH - 1
                dh, dw = s // KK, s % KK
                lhsT = kt[:, c * KCOLS + s * 128:c * KCOLS + (s + 1) * 128]
                for n in range(NTILES):
                    rhs = x3[c][:, n * ROWS_PER_TILE + dh:
                                n * ROWS_PER_TILE + dh + ROWS_PER_TILE,
                                dw:dw + W]
                    nc.tensor.matmul(cps[n][:], lhsT, rhs,
                                     start=False, stop=True)
                    use_dve = pingpong and (n % 2 == 1)
                    dst = o[:, n * NFREE:(n + 1) * NFREE]
                    if use_dve:
                        nc.vector.tensor_copy(out=dst, in_=cps[n][:])
                    else:
                        nc.scalar.activation(out=dst, in_=cps[n][:],
                                             func=Act.Copy)
                    if last:
                        # per-tile stores so the epilogue tail is one small
                        # DMA behind the final drain, not one big transfer
                        nc.sync.dma_start(
                            out=out_d[b, :,
                                      n * ROWS_PER_TILE:(n + 1) * ROWS_PER_TILE,
                                      :],
                            in_=o[:, n * NFREE:(n + 1) * NFREE])
                if not last:
                    nc.sync.dma_start(out=out_d[b], in_=o[:])

            # ---- software-pipelined emission ------------------------------
            # Sync-queue FIFO order gives DMA priority: watbat, xp(0), bank,
            # xp(1), xp(2), then per-iteration xp prefetch 3 samples ahead
            # behind each sample's output store. routing(b) is emitted with
            # gap(b) -- a full iteration before mix(b) -- so on the DVE queue
            # mix(b+2) sits right behind conv(b)'s ping-pong drains with its
            # sigmoid dependency long resolved, and the drains run the moment
            # conv(b) ends (PSUM recycling never waits on mix work).
            xps, gqs, kts = {}, {}, {}
            xps[0] = emit_xp(0)
            gqs[0] = emit_gap(0, xps[0])
            emit_bank_dma()
            emit_routing(0, gqs[0])
            kts[0] = emit_mix(0, [(0, 384), (384, 1152), (1152, KWID)])
            xps[1] = emit_xp(1)
            gqs[1] = emit_gap(1, xps[1])
            emit_routing(1, gqs[1])
            kts[1] = emit_mix(1, [(0, 1152), (1152, KWID)])
            xps[2] = emit_xp(2)
            gqs[2] = emit_gap(2, xps[2])
            emit_routing(2, gqs[2])
            for b in range(S):
                emit_conv(b, xps.pop(b), kts.pop(b), pingpong=(b > 0),
                          last=(b == S - 1))
                if b + 2 < S:
                    kts[b + 2] = emit_mix(b + 2, [(0, KWID)])
                if b + 3 < S:
                    xps[b + 3] = emit_xp(b + 3)
                    gqs[b + 3] = emit_gap(b + 3, xps[b + 3])
                    emit_routing(b + 3, gqs.pop(b + 3))

    nc.compile()
    return nc


def _prep_core_inputs(x, convs, W_att, b_att):
    """Host-side shard/layout prep. Returns list of 8 per-core input dicts."""
    import ml_dtypes
    f32 = np.float32
    bf16 = ml_dtypes.bfloat16
    # padded input, cin split into 2 chunks of 128
    xpad = np.zeros((B, CHUNKS, 128, HP, WP), dtype=bf16)
    xpad[:, :, :, 1:H + 1, 1:W + 1] = np.ascontiguousarray(x, dtype=f32).reshape(
        B, CHUNKS, 128, H, W).astype(bf16)
    xpad = xpad.reshape(B, CHUNKS, 128, PHW)

    # bank[half][c, g, p, e, kw*128 + m] = convs[e, half*128+m, c*128+p, g, kw]
    cv = np.ascontiguousarray(convs, dtype=f32).reshape(
        E, 2, MHALF, CHUNKS, 128, KK, KK)
    bank_halves = [
        np.ascontiguousarray(cv[:, h].transpose(2, 4, 3, 0, 5, 1).reshape(
            CHUNKS, 3, 128, E, 3 * 128)).astype(bf16)
        for h in range(2)
    ]

    watt = (np.asarray(W_att, dtype=f32).T / f32(H * W)).astype(f32)  # [CIN, E]
    watbat = np.empty((128, 3 * E), dtype=f32)
    watbat[:, 0:E] = watt[:128]
    watbat[:, E:2 * E] = watt[128:]
    # b_att/128 on every partition: the routing partition all-reduce sums it
    # back to b_att, so no separate bias add is needed
    watbat[:, 2 * E:3 * E] = np.broadcast_to(
        np.asarray(b_att, dtype=f32) / f32(128), (128, E))

    in_maps = []
    for k in range(NCORES):
        pair, half = k // 2, k % 2
        sl = slice(pair * SAMPLES_PER_CORE, (pair + 1) * SAMPLES_PER_CORE)
        in_maps.append({
            "xpad": np.ascontiguousarray(xpad[sl]),
            "bank": bank_halves[half],
            "watbat": watbat,
        })
    return in_maps


def _assemble_output(results):
    out = np.empty((B, COUT, H, W), dtype=np.float32)
    for k in range(NCORES):
        pair, half = k // 2, k % 2
        sl = slice(pair * SAMPLES_PER_CORE, (pair + 1) * SAMPLES_PER_CORE)
        out[sl, half * MHALF:(half + 1) * MHALF] = np.asarray(
            results[k]["out"], dtype=np.float32)
    return out


def kernel(x, convs, W_att, b_att):
    from concourse.bass_utils import run_bass_kernel_spmd

    if "nc" not in _cached:
        _cached["nc"] = _build_program()
    in_maps = _prep_core_inputs(x, convs, W_att, b_att)
    res = run_bass_kernel_spmd(_cached["nc"], in_maps, core_ids=list(range(NCORES)))
    return _assemble_output(res.results)


# revision 9
# speedup vs baseline: 1.0826x; 1.0297x over previous
"""CondConv (per-sample routed 3x3 conv) on 8 Trainium2 NeuronCores.

Reference computation (all fp32):
    gap     = mean(x, axis=(2,3))                    [B, CIN]
    routing = sigmoid(gap @ W_att.T + b_att)         [B, E]
    ker     = einsum('be,eoihw->boihw', routing, convs)
    out[b]  = conv2d(x[b], ker[b], stride 1, pad 1)  [B, COUT, 56, 56]

Sharding (B=32, COUT=256 across 8 cores): 4 core-pairs; pair p owns
samples 8p..8p+7 (batch data-parallel), and within a pair each core
computes one half of COUT (128 channels).

Per-core program (SPMD), bf16 datapath, fp32 PSUM accumulation:
  - expert bank resident in ONE SBUF tile [128cin, E*2304] so the whole
    bank loads as 6 large DMAs (vs 16 small ones); DMA order is
    xp(0) -> bank -> xp(1) -> xp(2) so sample 0's GAP/routing overlaps
    the bank load and the first matmul fires as early as possible.
  - routing on ScalarE(GAP accum + sigmoid)/DVE/GPSIMD; TensorE queue
    stays pure conv.
  - kernel mix on DVE as 8 tensor_scalar mults (4x bf16 mode) + 7
    tensor_tensor adds (2x bf16 mode) ~15.6us/sample, well under PE's
    ~25us/sample -- STT (no fast mode) would be 20.4us and starve the
    pipeline during the prologue. Samples 0/1 mix in column groups so
    conv(0) starts after only the first group.
  - conv: per sample 2chunk*9shift*7tile accumulating bf16 matmuls
    (N=448) into 7 PSUM tiles drawn from an 8-buffer rotating pool;
    the last accumulation round interleaves drains (ScalarE/DVE
    ping-pong) right behind each tile's final matmul so the next
    sample's matmuls never wait on PSUM recycling.
  - output: drains collect into one [128, 3136] SBUF tile, stored with
    a single DMA per sample (last sample: per-tile DMAs to cut the
    epilogue tail).
"""

import numpy as np

B, CIN, H, W = 32, 256, 56, 56
COUT, KK, E = 256, 3, 8
HP, WP = H + 2, W + 2          # zero-padded input plane
PHW = HP * WP                  # 3364
NSH = KK * KK                  # 9 shifts
CHUNKS = 2                     # CIN = 2 * 128
MHALF = COUT // 2              # couts per core
ROWS_PER_TILE = 8              # output rows per matmul tile
NTILES = H // ROWS_PER_TILE    # 7
NFREE = ROWS_PER_TILE * W      # 448
NCORES = 8
SAMPLES_PER_CORE = B // (NCORES // 2)  # 8
KCOLS = NSH * 128              # 1152 kernel cols per chunk
KWID = CHUNKS * KCOLS          # 2304 kernel cols per sample

_cached = {}


def _build_program():
    import concourse.bacc as bacc
    import concourse.bass_isa as bass_isa
    import concourse.mybir as mybir
    from concourse.tile import TileContext

    f32 = mybir.dt.float32
    bf16 = mybir.dt.bfloat16
    Alu = mybir.AluOpType
    Act = mybir.ActivationFunctionType

    nc = bacc.Bacc(None, target_bir_lowering=False)

    S = SAMPLES_PER_CORE
    xpad_d = nc.declare_dram_parameter(
        "xpad", [S, CHUNKS, 128, PHW], bf16, isOutput=False)
    bank_d = nc.declare_dram_parameter(
        "bank", [CHUNKS, 3, 128, E, 3 * 128], bf16, isOutput=False)
    watbat_d = nc.declare_dram_parameter("watbat", [128, 3 * E], f32,
                                         isOutput=False)
    out_d = nc.declare_dram_parameter(
        "out", [S, MHALF, H, W], bf16, isOutput=True)

    with TileContext(nc) as tc:
        with (
            tc.tile_pool(name="resident", bufs=1) as res_pool,
            tc.tile_pool(name="xp", bufs=3) as xp_pool,
            tc.tile_pool(name="kt", bufs=3) as kt_pool,
            tc.tile_pool(name="mixt", bufs=1) as t_pool,
            tc.tile_pool(name="mixu", bufs=2) as u_pool,
            tc.tile_pool(name="small", bufs=3) as small_pool,
            tc.tile_pool(name="outsb", bufs=2) as out_pool,
            tc.tile_pool(name="cpsum", bufs=8, space="PSUM") as cps_pool,
        ):
            # ---- small resident tiles -------------------------------------
            watbat_sb = res_pool.tile([128, 3 * E], f32, name="watbat",
                                      tag="watbat")
            nc.sync.dma_start(out=watbat_sb[:], in_=watbat_d[:])
            # warm the ScalarE activation tables (Copy+Sigmoid) at t~0 --
            # dep-free via a GpSimd memset -- so neither the routing sigmoid
            # nor the first drain pays the table load on the critical path
            warm = small_pool.tile([128, 1], f32, name="warm", tag="warm")
            nc.gpsimd.memset(warm[:], 0.0)
            nc.scalar.activation(out=warm[:], in_=warm[:], func=Act.Copy)
            nc.scalar.activation(out=warm[:], in_=warm[:], func=Act.Sigmoid)
            # broadcast routing weights: scal[:, 8*b+e] = r_be on every
            # partition
            scal_sb = res_pool.tile([128, S * E], f32, name="scal", tag="scal")
            # whole expert bank in one tile: col = e*2304 + c*1152 + s*128 + m
            bank_sb = res_pool.tile([128, E * KWID], bf16, name="bank",
                                    tag="bank")

            def emit_bank_dma():
                v = bank_sb.rearrange("p (e q) -> p e q", e=E)
                for c in range(CHUNKS):
                    for g in range(3):
                        a = c * KCOLS + g * 384
                        nc.sync.dma_start(out=v[:, :, a:a + 384],
                                          in_=bank_d[c, g])

            def emit_xp(b, split=1):
                xp = []
                for c in range(CHUNKS):
                    t = xp_pool.tile([128, PHW], bf16, name=f"xp{c}",
                                     tag=f"xp{c}")
                    if split == 1:
                        nc.sync.dma_start(out=t[:], in_=xpad_d[b, c])
                    else:
                        piece = PHW // split
                        for j in range(split):
                            sl = slice(j * piece, (j + 1) * piece)
                            nc.sync.dma_start(out=t[:, sl],
                                              in_=xpad_d[b, c, :, sl])
                    xp.append(t)
                return xp

            def emit_gap(b, xp, split=1):
                """GAP via ScalarE in-place Copy whose accum_out yields the
                per-partition row sums (bf16 in, f32 accumulate)."""
                gq = []
                for c in range(CHUNKS):
                    pieces = []
                    w = PHW // split
                    for j in range(split):
                        sl = slice(j * w, (j + 1) * w)
                        g = small_pool.tile([128, 1], f32, name=f"gh{c}_{j}",
                                            tag=f"gh{c}_{j}")
                        nc.scalar.activation(out=xp[c][:, sl],
                                             in_=xp[c][:, sl],
                                             func=Act.Copy, accum_out=g[:])
                        pieces.append(g)
                    gq.append(pieces)
                return gq

            def emit_routing(b, gq):
                """logits[e] = sum_cin gap[cin]*W_att[e,cin]/3136 + b_att[e]
                (1/3136 folded into watbat host-side; b_att/128 lives in
                watbat[:, 2E:3E] so the partition all-reduce sums it back to
                b_att -- no separate bias add); sigmoid -> scal_sb."""
                gsum = []
                for c in range(CHUNKS):
                    if len(gq[c]) == 1:
                        gsum.append(gq[c][0])
                    else:
                        g = small_pool.tile([128, 1], f32, name=f"gs{c}",
                                            tag=f"gs{c}")
                        nc.vector.tensor_add(out=g[:], in0=gq[c][0][:],
                                             in1=gq[c][1][:])
                        gsum.append(g)
                t0 = small_pool.tile([128, E], f32, name="t0", tag="t0")
                nc.vector.scalar_tensor_tensor(
                    out=t0[:], in0=watbat_sb[:, 0:E],
                    scalar=gsum[0][:, 0:1],
                    in1=watbat_sb[:, 2 * E:3 * E], op0=Alu.mult, op1=Alu.add)
                t1 = small_pool.tile([128, E], f32, name="t1", tag="t1")
                nc.vector.scalar_tensor_tensor(
                    out=t1[:], in0=watbat_sb[:, E:2 * E],
                    scalar=gsum[1][:, 0:1],
                    in1=t0[:], op0=Alu.mult, op1=Alu.add)
                red = small_pool.tile([128, E], f32, name="red", tag="red")
                nc.gpsimd.partition_all_reduce(red[:], t1[:], channels=128,
                                               reduce_op=bass_isa.ReduceOp.add)
                nc.scalar.activation(out=scal_sb[:, b * E:(b + 1) * E],
                                     in_=red[:], func=Act.Sigmoid)

            def emit_mix(b, groups):
                """Mix sample b's kernel on DVE with fast bf16 modes:
                t_e = r_be * bank_e   (tensor_scalar, 4x bf16)
                kt  = tree-sum(t_e)   (tensor_tensor adds, 2x bf16)
                `groups` splits the 2304 free cols so early groups unblock
                conv while later bank columns may still be in DMA flight.
                """
                kt = kt_pool.tile([128, KWID], bf16, name="kt", tag="kt")
                ts = [t_pool.tile([128, KWID], bf16, name=f"t{e}",
                                  tag=f"t{e}") for e in range(E)]
                u = [u_pool.tile([128, KWID], bf16, name=f"u{i}", tag=f"u{i}")
                     for i in range(6)]
                r = lambda e: scal_sb[:, b * E + e:b * E + e + 1]
                for (a, z) in groups:
                    for e in range(E):
                        nc.vector.tensor_scalar_mul(
                            out=ts[e][:, a:z],
                            in0=bank_sb[:, e * KWID + a:e * KWID + z],
                            scalar1=r(e))
                    pairs = [(u[0], ts[0], ts[1]), (u[1], ts[2], ts[3]),
                             (u[2], ts[4], ts[5]), (u[3], ts[6], ts[7]),
                             (u[4], u[0], u[1]), (u[5], u[2], u[3]),
                             (kt, u[4], u[5])]
                    for (o, i0, i1) in pairs:
                        nc.vector.tensor_add(out=o[:, a:z], in0=i0[:, a:z],
                                             in1=i1[:, a:z])
                return kt

            def emit_conv(b, xp, kt, dense, last):
                """Conv for sample b: accumulate 2c*9shift into 7 PSUM tiles
                from the 8-buffer rotating pool.

                dense=True (samples 1+): tiles 0,2,4 run all 18 rounds first
                (stopping 10-20us before sample end, DVE-drained at leisure),
                then tiles 1,3,5,6 round-major with ScalarE drains at sample
                end. With the +7 bank rotation, every drain the NEXT sample
                needs within ~7us comes from a dense tile that stopped long
                ago, so PSUM recycling never stalls the PE regardless of how
                the Tile scheduler orders drains among mix work.

                dense=False (sample 0): plain round-major, all drains on
                ScalarE -- required because sample 0's kt is mixed in column
                groups and round-major consumes kt shift by shift."""
                cps = [cps_pool.tile([128, NFREE], f32, name="cps", tag="cps")
                       for _ in range(NTILES)]
                x3 = [xp[c].rearrange("p (r q) -> p r q", q=WP)
                      for c in range(CHUNKS)]
                o = out_pool.tile([128, H * W], bf16, name="osb", tag="osb")

                def mm(n, c, s, first, stop):
                    dh, dw = s // KK, s % KK
                    lhsT = kt[:, c * KCOLS + s * 128:c * KCOLS + (s + 1) * 128]
                    rhs = x3[c][:, n * ROWS_PER_TILE + dh:
                                n * ROWS_PER_TILE + dh + ROWS_PER_TILE,
                                dw:dw + W]
                    nc.tensor.matmul(cps[n][:], lhsT, rhs,
                                     start=first, stop=stop)

                def drain(n, on_dve):
                    dst = o[:, n * NFREE:(n + 1) * NFREE]
                    if on_dve:
                        nc.vector.tensor_copy(out=dst, in_=cps[n][:])
                    else:
                        nc.scalar.activation(out=dst, in_=cps[n][:],
                                             func=Act.Copy)
                    if last:
                        # per-tile stores so the epilogue tail is one small
                        # DMA behind the final drain, not one big transfer
                        nc.sync.dma_start(
                            out=out_d[b, :,
                                      n * ROWS_PER_TILE:(n + 1) * ROWS_PER_TILE,
                                      :],
                            in_=o[:, n * NFREE:(n + 1) * NFREE])

                rounds = [(c, s) for c in range(CHUNKS) for s in range(NSH)]
                if dense:
                    for n in (0, 2, 4):
                        for i, (c, s) in enumerate(rounds):
                            mm(n, c, s, first=(i == 0), stop=(i == 17))
                        drain(n, on_dve=True)
                    late = (1, 3, 5, 6)
                else:
                    late = tuple(range(NTILES))
                for i, (c, s) in enumerate(rounds):
                    for n in late:
                        mm(n, c, s, first=(i == 0), stop=(i == 17))
                        if i == 17:
                            drain(n, on_dve=False)
                if not last:
                    nc.sync.dma_start(out=out_d[b], in_=o[:])

            # ---- software-pipelined emission ------------------------------
            # Sync-queue FIFO order gives DMA priority: watbat, xp(0), bank,
            # xp(1), xp(2), then per-iteration xp prefetch 3 samples ahead
            # behind each sample's output store. routing(b) is emitted with
            # gap(b) -- a full iteration before mix(b) -- so on the DVE queue
            # mix(b+2) sits right behind conv(b)'s ping-pong drains with its
            # sigmoid dependency long resolved, and the drains run the moment
            # conv(b) ends (PSUM recycling never waits on mix work).
            xps, gqs, kts = {}, {}, {}
            xps[0] = emit_xp(0)
            gqs[0] = emit_gap(0, xps[0])
            emit_bank_dma()
            emit_routing(0, gqs[0])
            kts[0] = emit_mix(0, [(0, 384), (384, 1152), (1152, KWID)])
            xps[1] = emit_xp(1)
            gqs[1] = emit_gap(1, xps[1])
            emit_routing(1, gqs[1])
            kts[1] = emit_mix(1, [(0, 1152), (1152, KWID)])
            xps[2] = emit_xp(2)
            gqs[2] = emit_gap(2, xps[2])
            emit_routing(2, gqs[2])
            for b in range(S):
                emit_conv(b, xps.pop(b), kts.pop(b), dense=(b > 0),
                          last=(b == S - 1))
                if b + 2 < S:
                    kts[b + 2] = emit_mix(b + 2, [(0, KWID)])
                if b + 3 < S:
                    xps[b + 3] = emit_xp(b + 3)
                    gqs[b + 3] = emit_gap(b + 3, xps[b + 3])
                    emit_routing(b + 3, gqs.pop(b + 3))

    nc.compile()
    return nc


def _prep_core_inputs(x, convs, W_att, b_att):
    """Host-side shard/layout prep. Returns list of 8 per-core input dicts."""
    import ml_dtypes
    f32 = np.float32
    bf16 = ml_dtypes.bfloat16
    # padded input, cin split into 2 chunks of 128
    xpad = np.zeros((B, CHUNKS, 128, HP, WP), dtype=bf16)
    xpad[:, :, :, 1:H + 1, 1:W + 1] = np.ascontiguousarray(x, dtype=f32).reshape(
        B, CHUNKS, 128, H, W).astype(bf16)
    xpad = xpad.reshape(B, CHUNKS, 128, PHW)

    # bank[half][c, g, p, e, kw*128 + m] = convs[e, half*128+m, c*128+p, g, kw]
    cv = np.ascontiguousarray(convs, dtype=f32).reshape(
        E, 2, MHALF, CHUNKS, 128, KK, KK)
    bank_halves = [
        np.ascontiguousarray(cv[:, h].transpose(2, 4, 3, 0, 5, 1).reshape(
            CHUNKS, 3, 128, E, 3 * 128)).astype(bf16)
        for h in range(2)
    ]

    watt = (np.asarray(W_att, dtype=f32).T / f32(H * W)).astype(f32)  # [CIN, E]
    watbat = np.empty((128, 3 * E), dtype=f32)
    watbat[:, 0:E] = watt[:128]
    watbat[:, E:2 * E] = watt[128:]
    # b_att/128 on every partition: the routing partition all-reduce sums it
    # back to b_att, so no separate bias add is needed
    watbat[:, 2 * E:3 * E] = np.broadcast_to(
        np.asarray(b_att, dtype=f32) / f32(128), (128, E))

    in_maps = []
    for k in range(NCORES):
        pair, half = k // 2, k % 2
        sl = slice(pair * SAMPLES_PER_CORE, (pair + 1) * SAMPLES_PER_CORE)
        in_maps.append({
            "xpad": np.ascontiguousarray(xpad[sl]),
            "bank": bank_halves[half],
            "watbat": watbat,
        })
    return in_maps


def _assemble_output(results):
    out = np.empty((B, COUT, H, W), dtype=np.float32)
    for k in range(NCORES):
        pair, half = k // 2, k % 2
        sl = slice(pair * SAMPLES_PER_CORE, (pair + 1) * SAMPLES_PER_CORE)
        out[sl, half * MHALF:(half + 1) * MHALF] = np.asarray(
            results[k]["out"], dtype=np.float32)
    return out


def kernel(x, convs, W_att, b_att):
    from concourse.bass_utils import run_bass_kernel_spmd

    if "nc" not in _cached:
        _cached["nc"] = _build_program()
    in_maps = _prep_core_inputs(x, convs, W_att, b_att)
    res = run_bass_kernel_spmd(_cached["nc"], in_maps, core_ids=list(range(NCORES)))
    return _assemble_output(res.results)
